# revision 1
# baseline (speedup 1.0000x reference)
"""Trainium2 Bass kernel v2 for conv-qkv rank-1 attention.

out = gamma * q * sum((k+bk)*(v+bv)) + x with q,k,v per-time-slice 3x3
convs of x [B=8, C=64, T=16, W=64, H=64]; data-parallel over B on 8 cores.

Key differences vs v1:
- bf16 matmul operands: K=64 bf16 matmuls run ~2x faster per moving row
  than fp32r (measured ~60ns vs 139ns per N=512 matmul).
- Conv bias handling off the PE: k-bias rides the DVE k*v op (op0=add),
  v-bias rides the PSUM->SBUF evacuation, q keeps a 10th ones-tap.
- x is pre-cast to bf16 on the host and loaded with two contiguous DMAs
  per pair, then pad-copied into the ringed conv tile by vector/scalar/
  gpsimd (4B-aligned interior) instead of 4096 x 256B strided descriptors.
- Final out = q*s + x fused in one 128-partition DVE pass per block.

Structure (m64): six M=64 chains per block round-robin per tap —
measured ~76ns/matmul effective (2-deep PE pipelining; M=128 chains do
NOT pipeline and run 140ns). Q=[q_t|q_t1] aligned, K/V swapped; the q
conv lags one pair so the final pass reads q straight from PSUM once
s is known, with 4 rotating Q banks riding out the s-reduce latency.
"""

import os
from contextlib import ExitStack

import numpy as np
import ml_dtypes

import concourse.bacc as bacc
import concourse.bass as bass
import concourse.mybir as mybir
import concourse.tile as tile
from concourse import bass_utils

F32 = mybir.dt.float32
BF16 = mybir.dt.bfloat16
ALU = mybir.AluOpType

B, C, T, W, H = 8, 64, 16, 64, 64
WP, HP = W + 2, H + 4  # 2-col left pad keeps bf16 copies 4B-aligned
NPAIR = int(os.environ.get("BASS_NPAIR", T // 2))
RB = 8
NBLK = W // RB
BN = RB * H  # 512

CFG = "m64"
BF = ml_dtypes.bfloat16


def _taps(w):  # [O, I, 1, 3, 3] -> [I, 9, O]
    return np.ascontiguousarray(
        np.asarray(w, np.float32).reshape(C, C, 9).transpose(1, 2, 0))


def _pack_weights(wq, wk, wv, bq):
    tq, tk, tv = _taps(wq), _taps(wk), _taps(wv)
    if CFG == "alpha":
        # cols 0:128 = [Wq|Wv] (A1, rows 0-63); 128:256 = [Wv|Wq] (A2,
        # rows 64-127); 256:320 = Wk (both halves). tap 9 = q bias row.
        wp = np.zeros((128, 10, 320), np.float32)
        wp[0:64, 0:9, 0:64] = tq
        wp[0:64, 0:9, 64:128] = tv
        wp[64:128, 0:9, 128:192] = tv
        wp[64:128, 0:9, 192:256] = tq
        wp[0:64, 0:9, 256:320] = tk
        wp[64:128, 0:9, 256:320] = tk
        wp[0, 9, 0:64] = bq
        wp[64, 9, 192:256] = bq
    else:
        # cols 0:64 = Wq (tap 9 = bias), 64:128 = Wk, 128:192 = Wv
        wp = np.zeros((128, 10, 192), np.float32)
        for h in (0, 64):
            wp[h:h + 64, 0:9, 0:64] = tq
            wp[h:h + 64, 0:9, 64:128] = tk
            wp[h:h + 64, 0:9, 128:192] = tv
        wp[0, 9, 0:64] = bq
        wp[64, 9, 0:64] = bq
    return wp.astype(BF)


def _emit(nc, tc, x_d, w_d, biases_d, ones_d, out_d, ctx):
    NCOL = 320 if CFG == "alpha" else 192
    const = ctx.enter_context(tc.tile_pool(name="const", bufs=1))
    state = ctx.enter_context(tc.tile_pool(name="state", bufs=1))
    vpool = ctx.enter_context(tc.tile_pool(name="vpool", bufs=3))
    # alpha uses per-parity tags (A10/A11/A20/A21, K0/K1) with bufs=1:
    # 4 + 2 = 6 PSUM banks; m64 uses single tags with bufs=2: 6 banks.
    nbuf = 1 if CFG == "alpha" else 2
    pA = ctx.enter_context(
        tc.tile_pool(name="pA", bufs=nbuf, space=bass.MemorySpace.PSUM))
    pK = ctx.enter_context(
        tc.tile_pool(name="pK", bufs=nbuf, space=bass.MemorySpace.PSUM))

    w_t = const.tile([128, 10, NCOL], BF16, tag="w")
    ones_t = const.tile([128, RB, H], BF16, tag="ones")
    bias_t = const.tile([128, 3], F32, tag="biases")  # cols: bk, bv, gamma
    nc.sync.dma_start(w_t[:], w_d[:])
    nc.sync.dma_start(ones_t[:], ones_d[:])
    nc.sync.dma_start(bias_t[:], biases_d[:])
    bk2 = bias_t[:, 0:1]
    gam = bias_t[:, 2:3]

    NXP = 4
    xp = [state.tile([128, WP, HP], BF16, tag=f"xp{i}", name=f"xp{i}")
          for i in range(NXP)]
    xc = [state.tile([128, W, H], BF16, tag=f"xc{i}", name=f"xc{i}")
          for i in range(2)]
    qs = [state.tile([128, W, H], F32, tag=f"qs{i}", name=f"qs{i}")
          for i in range(2)]
    ot = [state.tile([128, W, H], BF16, tag=f"ot{i}", name=f"ot{i}")
          for i in range(2)]
    scr = state.tile([128, RB, H], BF16, tag="scr")
    sS = [state.tile([128, NBLK], F32, tag=f"sS{i}", name=f"sS{i}")
          for i in range(2)]
    rS = [state.tile([128, 2], F32, tag=f"rS{i}", name=f"rS{i}")
          for i in range(2)]
    sgam = [state.tile([128, 1], F32, tag=f"sg{i}", name=f"sg{i}")
            for i in range(2)]

    # zero the pad rings once, split across engines to shorten the head
    for i, t_ in enumerate(xp):
        (nc.gpsimd if i % 2 else nc.vector).memset(t_[:, :, :], 0.0)

    def load_xp(p):
        # contiguous bf16 load of host-precast x, then pad-copy into the
        # ringed tile split across three engines (ring stays zero)
        c_ = xc[p % 2]
        nc.sync.dma_start(c_[0:64], x_d[:, 2 * p])
        nc.sync.dma_start(c_[64:128], x_d[:, 2 * p + 1])
        t_ = xp[p % NXP]
        nc.vector.tensor_copy(t_[:, 1:45, 2:2 + H], c_[:, 0:44, :])
        nc.scalar.copy(t_[:, 45:59, 2:2 + H], c_[:, 44:58, :])
        nc.gpsimd.tensor_copy(t_[:, 59:65, 2:2 + H], c_[:, 58:64, :])

    def rhs(t_, half, tap, j):
        if tap == 9:
            return ones_t[64 * half:64 * half + 64]
        dy, dx = tap // 3, tap % 3
        return t_[64 * half:64 * half + 64, j * RB + dy:j * RB + dy + RB,
                  dx + 1:dx + 1 + H]

    def chain_tap(t_, j, half, cols, tap, taps, out_ap, tpos):
        if tap >= taps:
            return
        nc.tensor.matmul(
            out_ap, w_t[64 * half:64 * half + 64, tap, cols[0]:cols[1]],
            rhs(t_, half, tap, j), start=(tap == 0),
            stop=(tap == taps - 1), tile_position=tpos)

    def s_finalize(w, swap):
        r = rS[w % 2]
        nc.vector.reduce_sum(r[:, 0:1], sS[w % 2][:, :],
                             axis=mybir.AxisListType.X)
        if swap:
            # sS halves are [s_t1 | s_t]: swap before scaling by gamma
            nc.scalar.dma_start(r[0:64, 1:2], r[64:128, 0:1])
            nc.scalar.dma_start(r[64:128, 1:2], r[0:64, 0:1])
            nc.vector.tensor_scalar_mul(sgam[w % 2][:, :], r[:, 1:2], gam)
        else:
            nc.vector.tensor_scalar_mul(sgam[w % 2][:, :], r[:, 0:1], gam)

    def fs(w, j, in0):
        # out = q * sgam + x, one 128-partition pass
        nc.vector.scalar_tensor_tensor(
            out=ot[w % 2][:, j * RB:(j + 1) * RB, :], in0=in0,
            scalar=sgam[w % 2][:, 0:1],
            in1=xp[w % NXP][:, 1 + j * RB:1 + (j + 1) * RB, 2:2 + H],
            op0=ALU.mult, op1=ALU.add)

    def kv_stt(w, j, kb, vsb):
        nc.vector.scalar_tensor_tensor(
            out=scr[:], in0=kb[:], scalar=bk2, in1=vsb[:],
            op0=ALU.add, op1=ALU.mult, accum_out=sS[w % 2][:, j:j + 1])

    def store_out(w):
        nc.gpsimd.dma_start(out_d[:, 2 * w], ot[w % 2][0:64])
        nc.gpsimd.dma_start(out_d[:, 2 * w + 1], ot[w % 2][64:128])

    load_xp(0)
    if NPAIR > 1:
        load_xp(1)

    if CFG == "alpha":
        for w in range(NPAIR):
            if w + 2 < NPAIR:
                load_xp(w + 2)
            t_ = xp[w % NXP]
            qs_, ot_ = qs[w % 2], ot[w % 2]
            vsbs = []
            # A phase: M=128 chains [q_t|v_t] and [v_t1|q_t1], interleaved
            # across block pairs for 4 independent accumulation streams
            for j0 in range(0, NBLK, 2):
                banks = []
                for j in (j0, j0 + 1):
                    a1 = pA.tile([128, RB, H], F32, tag=f"A1{j % 2}", name="a1")
                    a2 = pA.tile([128, RB, H], F32, tag=f"A2{j % 2}", name="a2")
                    banks.append((j, a1, a2))
                for tap in range(10):
                    for (j, a1, a2) in banks:
                        chain_tap(t_, j, 0, (0, 128), tap, 10, a1[:], (0, 0))
                        chain_tap(t_, j, 1, (128, 256), tap, 10, a2[:],
                                  (64, 0))
                for (j, a1, a2) in banks:
                    vsb = vpool.tile([128, RB, H], BF16, tag="vsb", name="vsb")
                    jsl = slice(j * RB, (j + 1) * RB)
                    nc.scalar.copy(qs_[0:64, jsl, :], a1[0:64])
                    nc.scalar.copy(qs_[64:128, jsl, :], a2[64:128])
                    nc.scalar.add(vsb[64:128], a1[64:128],
                                  bias_t[64:128, 1:2])
                    nc.vector.tensor_scalar_add(vsb[0:64], a2[0:64],
                                                bias_t[0:64, 1:2])
                    vsbs.append(vsb)
            # K phase: M=64 chains into swapped banks [k_t1 | k_t],
            # 4 half-bank streams across block pairs
            for j0 in range(0, NBLK, 2):
                kbs = [(j, pK.tile([128, RB, H], F32, tag=f"K{j % 2}", name="kb"))
                       for j in (j0, j0 + 1)]
                for tap in range(9):
                    for (j, kb) in kbs:
                        chain_tap(t_, j, 1, (256, 320), tap, 9, kb[0:64],
                                  (64, 0))
                        chain_tap(t_, j, 0, (256, 320), tap, 9, kb[64:128],
                                  (0, 64))
                for (j, kb) in kbs:
                    kv_stt(w, j, kb, vsbs[j])
            s_finalize(w, swap=True)
            for j in range(NBLK):
                fs(w, j, qs_[:, j * RB:(j + 1) * RB, :])
            store_out(w)
    else:
        pQ = ctx.enter_context(
            tc.tile_pool(name="pQ", bufs=4, space=bass.MemorySpace.PSUM))
        for w in range(NPAIR + 1):
            if w + 2 < NPAIR:
                load_xp(w + 2)
            tq_ = xp[(w - 1) % NXP]
            tkv_ = xp[w % NXP]
            # Merged per-block emission, six chains round-robin per tap:
            # [K-lo, K-hi, V-lo, V-hi, Q-lo, Q-hi]. kv for pair w into
            # swapped banks K=[k_t1|k_t], V=[v_t1|v_t]; q for pair w-1
            # (PSUM-direct, consumed by fs once s(w-1) is ready).
            for j in range(NBLK):
                mms = []
                if w < NPAIR:
                    kb = pK.tile([128, RB, H], F32, tag="K", name="kb")
                    vb = pA.tile([128, RB, H], F32, tag="V", name="vb")
                    mms += [
                        (tkv_, 1, (64, 128), 9, kb[0:64], (64, 0)),
                        (tkv_, 0, (64, 128), 9, kb[64:128], (0, 64)),
                        (tkv_, 1, (128, 192), 9, vb[0:64], (64, 0)),
                        (tkv_, 0, (128, 192), 9, vb[64:128], (0, 64)),
                    ]
                if w > 0:
                    qb = pQ.tile([128, RB, H], F32, tag="Q", name="qb")
                    mms += [
                        (tq_, 0, (0, 64), 10, qb[0:64], (0, 0)),
                        (tq_, 1, (0, 64), 10, qb[64:128], (64, 64)),
                    ]
                for tap in range(10):
                    for (t_, half, cols, taps, out_ap, tpos) in mms:
                        chain_tap(t_, j, half, cols, tap, taps, out_ap, tpos)
                if w < NPAIR:
                    vsb = vpool.tile([128, RB, H], BF16, tag="vsb", name="vsb")
                    nc.scalar.add(vsb[:], vb[:], bias_t[:, 1:2])
                    kv_stt(w, j, kb, vsb)
                if w > 0:
                    fs(w - 1, j, qb[:])
            if w < NPAIR:
                s_finalize(w, swap=True)
            if w > 0:
                store_out(w - 1)


_CACHE = {}


def _build():
    if CFG in _CACHE:
        return _CACHE[CFG]
    nc = bacc.Bacc("TRN2", target_bir_lowering=False, debug=False,
                   enable_asserts=False, num_devices=8)
    NCOL = 320 if CFG == "alpha" else 192
    x_d = nc.dram_tensor("x", (C, T, W, H), BF16, kind="ExternalInput").ap()
    w_d = nc.dram_tensor("wpack", (128, 10, NCOL), BF16,
                         kind="ExternalInput").ap()
    biases_d = nc.dram_tensor("biases", (128, 3), F32,
                              kind="ExternalInput").ap()
    ones_d = nc.dram_tensor("ones", (128, RB, H), BF16,
                            kind="ExternalInput").ap()
    out_d = nc.dram_tensor("out", (C, T, W, H), F32,
                           kind="ExternalOutput").ap()
    with tile.TileContext(nc) as tc, ExitStack() as ctx:
        _emit(nc, tc, x_d, w_d, biases_d, ones_d, out_d, ctx)
    nc.compile()
    _CACHE[CFG] = nc
    return nc


_ONES = np.ones((128, RB, H), BF)


def run_spmd(x, wq, wk, wv, bq, bk, bv, gamma, trace=False, **kw):
    nc = _build()
    wp = _pack_weights(wq, wk, wv, np.asarray(bq, np.float32))
    biases = np.zeros((128, 3), np.float32)
    for h in (0, 64):
        biases[h:h + 64, 0] = np.asarray(bk, np.float32)
        biases[h:h + 64, 1] = np.asarray(bv, np.float32)
    biases[:, 2] = np.float32(np.asarray(gamma).reshape(-1)[0])
    x = np.asarray(x, np.float32)
    in_maps = [
        {"x": np.ascontiguousarray(x[b]).astype(BF), "wpack": wp,
         "biases": biases,
         "ones": _ONES}
        for b in range(B)
    ]
    res = bass_utils.run_bass_kernel_spmd(
        nc, in_maps, core_ids=list(range(B)), trace=trace, **kw)
    out = np.stack([res.results[b]["out"] for b in range(B)], axis=0)
    return out, res


def kernel(x, wq, wk, wv, bq, bk, bv, gamma):
    out, _ = run_spmd(x, wq, wk, wv, bq, bk, bv, gamma)
    return out



# revision 2
# speedup vs baseline: 1.0107x; 1.0107x over previous
"""Trainium2 Bass kernel v4.1 for conv-qkv rank-1 attention.

out = gamma * q * sum((k+bk)*(v+bv)) + x with q,k,v per-time-slice 3x3
convs of x [B=8, C=64, T=16, W=64, H=64]; data-parallel over B on 8 cores.

Structure: keep all four 64x64 PE quadrants streaming every cycle.
- QK phase: M=128 fused [Wq|Wk] chains, K=64. The two time slices of a
  pair run as row-concurrent chains (rows 0:64 stream x_t, rows 64:128
  stream x_t1) into separate PSUM banks [q|k]. 9 taps; q bias rides the
  PSUM->SBUF evacuation add.
- V phase: 4 quadrant-concurrent M=64 K=64 chains covering two blocks
  x two slices at tile positions (0,0),(0,64),(64,0),(64,64).
- Per 2-block group: 36 QK MMs (2-concurrent) + 36 V MMs (4-concurrent)
  = 27 N<=512 slot-times = 100% array-utilization ideal.
- No padded x tile: conv zero-padding is realized by edge-TRUNCATED
  matmul APs (a tap whose input row/col falls outside [0,64) simply
  skips those output positions; tap (1,1) has full coverage and runs
  first with start=True so partial taps accumulate on clean PSUM).
  x DMAs directly into contiguous [128, W, H] tiles - no pad-copy.
- q is evacuated to SBUF bf16 (+bq) on vector/gpsimd; k (+bk) on scalar
  aligned with the V banks; kv product + position-sum via one 128-wide
  DVE STT (accum_out) per bank; fs of pair w runs during pair w+1.
"""

from contextlib import ExitStack

import numpy as np
import ml_dtypes

import concourse.bacc as bacc
import concourse.bass as bass
import concourse.mybir as mybir
import concourse.tile as tile
from concourse import bass_utils

F32 = mybir.dt.float32
BF16 = mybir.dt.bfloat16
ALU = mybir.AluOpType

B, C, T, W, H = 8, 64, 16, 64, 64
NPAIR = T // 2
RB = 8
NBLK = W // RB  # 8
NGRP = NBLK // 2  # 4
TAP_ORDER = [4, 0, 1, 2, 3, 5, 6, 7, 8]  # (1,1) first: full coverage

BF = ml_dtypes.bfloat16


def _taps(w):  # [O, I, 1, 3, 3] -> [I, 9, O]
    return np.ascontiguousarray(
        np.asarray(w, np.float32).reshape(C, C, 9).transpose(1, 2, 0))


def _pack_weights(wq, wk, wv):
    # cols 0:64 = Wq, 64:128 = Wk (contiguous [Wq|Wk] for fused M=128
    # chains), 128:192 = Wv
    tq, tk, tv = _taps(wq), _taps(wk), _taps(wv)
    wp = np.zeros((128, 9, 192), np.float32)
    for h in (0, 64):
        wp[h:h + 64, :, 0:64] = tq
        wp[h:h + 64, :, 64:128] = tk
        wp[h:h + 64, :, 128:192] = tv
    return wp.astype(BF)


def _emit(nc, tc, x_d, w_d, biases_d, out_d, ctx):
    const = ctx.enter_context(tc.tile_pool(name="const", bufs=1))
    state = ctx.enter_context(tc.tile_pool(name="state", bufs=1))
    kpool = ctx.enter_context(tc.tile_pool(name="kpool", bufs=2))
    # PSUM: 4 QK banks (bufs=1) + 2 V banks (bufs=2) = 8 banks exactly
    pQK = ctx.enter_context(
        tc.tile_pool(name="pQK", bufs=1, space=bass.MemorySpace.PSUM))
    pV = ctx.enter_context(
        tc.tile_pool(name="pV", bufs=2, space=bass.MemorySpace.PSUM))

    w_t = const.tile([128, 9, 192], BF16, tag="w")
    bias_t = const.tile([128, 4], F32, tag="biases")  # cols: bq, bk, bv, gam
    nc.sync.dma_start(w_t[:], w_d[:])
    nc.sync.dma_start(bias_t[:], biases_d[:])
    bq2 = bias_t[:, 0:1]
    bk2 = bias_t[:, 1:2]
    bv2 = bias_t[:, 2:3]
    gam = bias_t[:, 3:4]

    NXP = 4
    xp = [state.tile([128, W, H], BF16, tag=f"xp{i}", name=f"xp{i}")
          for i in range(NXP)]
    qs = [state.tile([128, W, H], BF16, tag=f"qs{i}", name=f"qs{i}")
          for i in range(2)]
    ot = [state.tile([128, W, H], BF16, tag=f"ot{i}", name=f"ot{i}")
          for i in range(2)]
    scr = [state.tile([128, RB, H], BF16, tag=f"scr{i}", name=f"scr{i}")
           for i in range(2)]
    sS = [state.tile([128, 2, NGRP], F32, tag=f"sS{i}", name=f"sS{i}")
          for i in range(2)]
    rS = [state.tile([128, 2], F32, tag=f"rS{i}", name=f"rS{i}")
          for i in range(2)]
    rSw = [state.tile([128, 2], F32, tag=f"rSw{i}", name=f"rSw{i}")
           for i in range(2)]
    sgam = [state.tile([128, 1], F32, tag=f"sg{i}", name=f"sg{i}")
            for i in range(2)]

    def load_xp(p):
        t_ = xp[p % NXP]
        nc.sync.dma_start(t_[0:64], x_d[:, 2 * p])
        nc.sync.dma_start(t_[64:128], x_d[:, 2 * p + 1])

    def mm_aps(t_, half, tap, j, bank_ap):
        # edge-truncated rhs/out APs realizing zero padding
        dy, dx = tap // 3, tap % 3
        r0 = j * RB + dy - 1          # first x row
        xr0, xr1 = max(r0, 0), min(r0 + RB, W)
        or0, or1 = xr0 - r0, xr1 - r0
        c0 = dx - 1
        xc0, xc1 = max(c0, 0), min(c0 + H, H)
        oc0, oc1 = xc0 - c0, xc1 - c0
        rhs = t_[64 * half:64 * half + 64, xr0:xr1, xc0:xc1]
        out = bank_ap[:, or0:or1, oc0:oc1]
        return out, rhs

    def fs(w, j):
        # out = q * sgam + x, one 128-partition bf16 pass
        jsl = slice(j * RB, (j + 1) * RB)
        nc.vector.scalar_tensor_tensor(
            out=ot[w % 2][:, jsl, :], in0=qs[w % 2][:, jsl, :],
            scalar=sgam[w % 2][:, 0:1], in1=xp[w % NXP][:, jsl, :],
            op0=ALU.mult, op1=ALU.add)

    def store_out(w):
        nc.gpsimd.dma_start(out_d[:, 2 * w], ot[w % 2][0:64])
        nc.scalar.dma_start(out_d[:, 2 * w + 1], ot[w % 2][64:128])

    load_xp(0)
    load_xp(1)

    for w in range(NPAIR + 1):
        if w + 2 < NPAIR:
            load_xp(w + 2)
        if w < NPAIR:
            t_ = xp[w % NXP]
            for m in range(NGRP):
                j0, j1 = 2 * m, 2 * m + 1
                qk = [[pQK.tile([128, RB, H], F32, tag=f"qk{a}{h}",
                                name=f"qk{a}{h}") for h in (0, 1)]
                      for a in (0, 1)]
                ksbA = kpool.tile([128, RB, H], BF16, tag="ksbA", name="ksbA")
                ksbB = kpool.tile([128, RB, H], BF16, tag="ksbB", name="ksbB")
                # QK phase: fused [q|k] M=128, row-concurrent over slices
                for a, j in ((0, j0), (1, j1)):
                    for ti, tap in enumerate(TAP_ORDER):
                        for h in (0, 1):
                            oap, rap = mm_aps(t_, h, tap, j, qk[a][h])
                            nc.tensor.matmul(
                                oap, w_t[64 * h:64 * h + 64, tap, 0:128],
                                rap, start=(ti == 0), stop=(ti == 8),
                                tile_position=(64 * h, 0))
                    # evacuate q (+bq, vector/gpsimd) and k (+bk, scalar,
                    # aligned with the V banks) as soon as the chain stops
                    jsl = slice(j * RB, (j + 1) * RB)
                    nc.vector.tensor_scalar_add(
                        qs[w % 2][0:64, jsl, :], qk[a][0][0:64], bq2[0:64])
                    nc.scalar.add(
                        qs[w % 2][64:128, jsl, :], qk[a][1][0:64],
                        bq2[64:128])
                    nc.scalar.add(ksbA[64 * a:64 * a + 64],
                                  qk[a][0][64:128], bk2[64 * a:64 * a + 64])
                    nc.scalar.add(ksbB[64 * a:64 * a + 64],
                                  qk[a][1][64:128], bk2[64 * a:64 * a + 64])
                # V phase: 4 quadrant-concurrent chains (2 blocks x 2 slices)
                va = pV.tile([128, RB, H], F32, tag="va", name="va")
                vb = pV.tile([128, RB, H], F32, tag="vb", name="vb")
                vspecs = ((va[0:64], 0, j0, (0, 0)),
                          (va[64:128], 0, j1, (0, 64)),
                          (vb[0:64], 1, j0, (64, 0)),
                          (vb[64:128], 1, j1, (64, 64)))
                for ti, tap in enumerate(TAP_ORDER):
                    for (oap_b, h, j, tp) in vspecs:
                        oap, rap = mm_aps(t_, h, tap, j, oap_b)
                        nc.tensor.matmul(
                            oap, w_t[64 * h:64 * h + 64, tap, 128:192],
                            rap, start=(ti == 0), stop=(ti == 8),
                            tile_position=tp)
                # kv product + position sum: sS[:, si, m] partitions 0:64 =
                # block j0 contribution, 64:128 = block j1
                for si, (vbank, ksb) in enumerate(((va, ksbA), (vb, ksbB))):
                    nc.vector.scalar_tensor_tensor(
                        out=scr[si][:], in0=vbank[:], scalar=bv2,
                        in1=ksb[:], op0=ALU.add, op1=ALU.mult,
                        accum_out=sS[w % 2][:, si, m:m + 1])
            # s finalize: reduce groups, fold block-parity halves (via a
            # cross-partition DMA swap; both-SBUF ops can't cross), scale
            r = rS[w % 2]
            r2 = rSw[w % 2]
            nc.vector.reduce_sum(r[:, 0:2], sS[w % 2][:, :, :],
                                 axis=mybir.AxisListType.X)
            nc.sync.dma_start(r2[0:64, 0:2], r[64:128, 0:2])
            nc.sync.dma_start(r2[64:128, 0:2], r[0:64, 0:2])
            nc.vector.tensor_tensor(out=r2[:, 0:2], in0=r[:, 0:2],
                                    in1=r2[:, 0:2], op=ALU.add)
            # sgam = gam * [s_t (col 0, lo) | s_t1 (col 1, hi)]
            nc.vector.tensor_scalar_mul(sgam[w % 2][0:64, 0:1],
                                        r2[0:64, 0:1], gam[0:64])
            nc.vector.tensor_scalar_mul(sgam[w % 2][64:128, 0:1],
                                        r2[64:128, 1:2], gam[64:128])
        if w > 0:
            for j in range(NBLK):
                fs(w - 1, j)
            store_out(w - 1)


_CACHE = {}


def _build():
    if "nc" in _CACHE:
        return _CACHE["nc"]
    nc = bacc.Bacc("TRN2", target_bir_lowering=False, debug=False,
                   enable_asserts=False, num_devices=8)
    x_d = nc.dram_tensor("x", (C, T, W, H), BF16, kind="ExternalInput").ap()
    w_d = nc.dram_tensor("wpack", (128, 9, 192), BF16,
                         kind="ExternalInput").ap()
    biases_d = nc.dram_tensor("biases", (128, 4), F32,
                              kind="ExternalInput").ap()
    out_d = nc.dram_tensor("out", (C, T, W, H), BF16,
                           kind="ExternalOutput").ap()
    with tile.TileContext(nc) as tc, ExitStack() as ctx:
        _emit(nc, tc, x_d, w_d, biases_d, out_d, ctx)
    nc.compile()
    _CACHE["nc"] = nc
    return nc


def run_spmd(x, wq, wk, wv, bq, bk, bv, gamma, trace=False, **kw):
    nc = _build()
    wp = _pack_weights(wq, wk, wv)
    biases = np.zeros((128, 4), np.float32)
    for h in (0, 64):
        biases[h:h + 64, 0] = np.asarray(bq, np.float32)
        biases[h:h + 64, 1] = np.asarray(bk, np.float32)
        biases[h:h + 64, 2] = np.asarray(bv, np.float32)
    biases[:, 3] = np.float32(np.asarray(gamma).reshape(-1)[0])
    x = np.asarray(x, np.float32)
    in_maps = [
        {"x": np.ascontiguousarray(x[b]).astype(BF), "wpack": wp,
         "biases": biases}
        for b in range(B)
    ]
    res = bass_utils.run_bass_kernel_spmd(
        nc, in_maps, core_ids=list(range(B)), trace=trace, **kw)
    out = np.stack([np.asarray(res.results[b]["out"], np.float32)
                    for b in range(B)], axis=0)
    return out, res


def kernel(x, wq, wk, wv, bq, bk, bv, gamma):
    out, _ = run_spmd(x, wq, wk, wv, bq, bk, bv, gamma)
    return out


# revision 3
# speedup vs baseline: 1.0110x; 1.0003x over previous
"""Trainium2 Bass kernel v4.1 for conv-qkv rank-1 attention.

out = gamma * q * sum((k+bk)*(v+bv)) + x with q,k,v per-time-slice 3x3
convs of x [B=8, C=64, T=16, W=64, H=64]; data-parallel over B on 8 cores.

Structure: keep all four 64x64 PE quadrants streaming every cycle.
- QK phase: M=128 fused [Wq|Wk] chains, K=64. The two time slices of a
  pair run as row-concurrent chains (rows 0:64 stream x_t, rows 64:128
  stream x_t1) into separate PSUM banks [q|k]. 9 taps; q bias rides the
  PSUM->SBUF evacuation add.
- V phase: 4 quadrant-concurrent M=64 K=64 chains covering two blocks
  x two slices at tile positions (0,0),(0,64),(64,0),(64,64).
- Per 2-block group: 36 QK MMs (2-concurrent) + 36 V MMs (4-concurrent)
  = 27 N<=512 slot-times = 100% array-utilization ideal.
- No padded x tile: conv zero-padding is realized by edge-TRUNCATED
  matmul APs (a tap whose input row/col falls outside [0,64) simply
  skips those output positions; tap (1,1) has full coverage and runs
  first with start=True so partial taps accumulate on clean PSUM).
  x DMAs directly into contiguous [128, W, H] tiles - no pad-copy.
- q is evacuated to SBUF bf16 (+bq) on vector/gpsimd; k (+bk) on scalar
  aligned with the V banks; kv product + position-sum via one 128-wide
  DVE STT (accum_out) per bank; fs of pair w runs during pair w+1.
"""

from contextlib import ExitStack

import numpy as np
import ml_dtypes

import concourse.bacc as bacc
import concourse.bass as bass
import concourse.mybir as mybir
import concourse.tile as tile
from concourse import bass_utils

F32 = mybir.dt.float32
BF16 = mybir.dt.bfloat16
ALU = mybir.AluOpType

B, C, T, W, H = 8, 64, 16, 64, 64
NPAIR = T // 2
RB = 8
NBLK = W // RB  # 8
NGRP = NBLK // 2  # 4
TAP_ORDER = [4, 0, 1, 2, 3, 5, 6, 7, 8]  # (1,1) first: full coverage

BF = ml_dtypes.bfloat16


def _taps(w):  # [O, I, 1, 3, 3] -> [I, 9, O]
    return np.ascontiguousarray(
        np.asarray(w, np.float32).reshape(C, C, 9).transpose(1, 2, 0))


def _pack_weights(wq, wk, wv):
    # cols 0:64 = Wq, 64:128 = Wk (contiguous [Wq|Wk] for fused M=128
    # chains), 128:192 = Wv
    tq, tk, tv = _taps(wq), _taps(wk), _taps(wv)
    wp = np.zeros((128, 9, 192), np.float32)
    for h in (0, 64):
        wp[h:h + 64, :, 0:64] = tq
        wp[h:h + 64, :, 64:128] = tk
        wp[h:h + 64, :, 128:192] = tv
    return wp.astype(BF)


def _emit(nc, tc, x_d, w_d, biases_d, out_d, ctx):
    const = ctx.enter_context(tc.tile_pool(name="const", bufs=1))
    state = ctx.enter_context(tc.tile_pool(name="state", bufs=1))
    kpool = ctx.enter_context(tc.tile_pool(name="kpool", bufs=2))
    # PSUM: 4 QK banks (bufs=1) + 2 V banks (bufs=2) = 8 banks exactly
    pQK = ctx.enter_context(
        tc.tile_pool(name="pQK", bufs=1, space=bass.MemorySpace.PSUM))
    pV = ctx.enter_context(
        tc.tile_pool(name="pV", bufs=2, space=bass.MemorySpace.PSUM))

    w_t = const.tile([128, 9, 192], BF16, tag="w")
    bias_t = const.tile([128, 4], F32, tag="biases")  # cols: bq, bk, bv, gam
    nc.sync.dma_start(w_t[:], w_d[:])
    nc.sync.dma_start(bias_t[:], biases_d[:])
    bq2 = bias_t[:, 0:1]
    bk2 = bias_t[:, 1:2]
    bv2 = bias_t[:, 2:3]
    gam = bias_t[:, 3:4]

    NXP = 4
    xp = [state.tile([128, W, H], BF16, tag=f"xp{i}", name=f"xp{i}")
          for i in range(NXP)]
    qs = [state.tile([128, W, H], BF16, tag=f"qs{i}", name=f"qs{i}")
          for i in range(2)]
    ot = [state.tile([128, W, H], BF16, tag=f"ot{i}", name=f"ot{i}")
          for i in range(2)]
    scr = [state.tile([128, RB, H], BF16, tag=f"scr{i}", name=f"scr{i}")
           for i in range(2)]
    sS = [state.tile([128, 2, NGRP], F32, tag=f"sS{i}", name=f"sS{i}")
          for i in range(2)]
    rS = [state.tile([128, 2], F32, tag=f"rS{i}", name=f"rS{i}")
          for i in range(2)]
    rSw = [state.tile([128, 2], F32, tag=f"rSw{i}", name=f"rSw{i}")
           for i in range(2)]
    sgam = [state.tile([128, 1], F32, tag=f"sg{i}", name=f"sg{i}")
            for i in range(2)]

    def load_xp(p):
        t_ = xp[p % NXP]
        nc.sync.dma_start(t_[0:64], x_d[:, 2 * p])
        nc.scalar.dma_start(t_[64:128], x_d[:, 2 * p + 1])

    def mm_aps(t_, half, tap, j, bank_ap):
        # edge-truncated rhs/out APs realizing zero padding
        dy, dx = tap // 3, tap % 3
        r0 = j * RB + dy - 1          # first x row
        xr0, xr1 = max(r0, 0), min(r0 + RB, W)
        or0, or1 = xr0 - r0, xr1 - r0
        c0 = dx - 1
        xc0, xc1 = max(c0, 0), min(c0 + H, H)
        oc0, oc1 = xc0 - c0, xc1 - c0
        rhs = t_[64 * half:64 * half + 64, xr0:xr1, xc0:xc1]
        out = bank_ap[:, or0:or1, oc0:oc1]
        return out, rhs

    def fs(w, j):
        # out = q * sgam + x, one 128-partition bf16 pass
        jsl = slice(j * RB, (j + 1) * RB)
        nc.vector.scalar_tensor_tensor(
            out=ot[w % 2][:, jsl, :], in0=qs[w % 2][:, jsl, :],
            scalar=sgam[w % 2][:, 0:1], in1=xp[w % NXP][:, jsl, :],
            op0=ALU.mult, op1=ALU.add)

    def store_half(w, lo):
        # store W-rows half as soon as its fs blocks are done (scalar's
        # hardware dynamic DMA queue; gpsimd's software DGE is ~6us/issue)
        wsl = slice(0, 32) if lo else slice(32, 64)
        nc.scalar.dma_start(out_d[:, 2 * w, wsl], ot[w % 2][0:64, wsl])
        nc.scalar.dma_start(out_d[:, 2 * w + 1, wsl], ot[w % 2][64:128, wsl])

    load_xp(0)
    load_xp(1)

    for w in range(NPAIR + 1):
        if w + 2 < NPAIR:
            load_xp(w + 2)
        if w < NPAIR:
            t_ = xp[w % NXP]
            for m in range(NGRP):
                j0, j1 = 2 * m, 2 * m + 1
                qk = [[pQK.tile([128, RB, H], F32, tag=f"qk{a}{h}",
                                name=f"qk{a}{h}") for h in (0, 1)]
                      for a in (0, 1)]
                ksbA = kpool.tile([128, RB, H], BF16, tag="ksbA", name="ksbA")
                ksbB = kpool.tile([128, RB, H], BF16, tag="ksbB", name="ksbB")
                # QK phase: fused [q|k] M=128, row-concurrent over slices
                for a, j in ((0, j0), (1, j1)):
                    for ti, tap in enumerate(TAP_ORDER):
                        for h in (0, 1):
                            oap, rap = mm_aps(t_, h, tap, j, qk[a][h])
                            nc.tensor.matmul(
                                oap, w_t[64 * h:64 * h + 64, tap, 0:128],
                                rap, start=(ti == 0), stop=(ti == 8),
                                tile_position=(64 * h, 0))
                    # evacuate q (+bq, vector/gpsimd) and k (+bk, scalar,
                    # aligned with the V banks) as soon as the chain stops
                    jsl = slice(j * RB, (j + 1) * RB)
                    nc.vector.tensor_scalar_add(
                        qs[w % 2][0:64, jsl, :], qk[a][0][0:64], bq2[0:64])
                    nc.scalar.add(
                        qs[w % 2][64:128, jsl, :], qk[a][1][0:64],
                        bq2[64:128])
                    nc.scalar.add(ksbA[64 * a:64 * a + 64],
                                  qk[a][0][64:128], bk2[64 * a:64 * a + 64])
                    nc.scalar.add(ksbB[64 * a:64 * a + 64],
                                  qk[a][1][64:128], bk2[64 * a:64 * a + 64])
                # V phase: 4 quadrant-concurrent chains (2 blocks x 2 slices)
                va = pV.tile([128, RB, H], F32, tag="va", name="va")
                vb = pV.tile([128, RB, H], F32, tag="vb", name="vb")
                vspecs = ((va[0:64], 0, j0, (0, 0)),
                          (va[64:128], 0, j1, (0, 64)),
                          (vb[0:64], 1, j0, (64, 0)),
                          (vb[64:128], 1, j1, (64, 64)))
                for ti, tap in enumerate(TAP_ORDER):
                    for (oap_b, h, j, tp) in vspecs:
                        oap, rap = mm_aps(t_, h, tap, j, oap_b)
                        nc.tensor.matmul(
                            oap, w_t[64 * h:64 * h + 64, tap, 128:192],
                            rap, start=(ti == 0), stop=(ti == 8),
                            tile_position=tp)
                # kv product + position sum: sS[:, si, m] partitions 0:64 =
                # block j0 contribution, 64:128 = block j1
                for si, (vbank, ksb) in enumerate(((va, ksbA), (vb, ksbB))):
                    nc.vector.scalar_tensor_tensor(
                        out=scr[si][:], in0=vbank[:], scalar=bv2,
                        in1=ksb[:], op0=ALU.add, op1=ALU.mult,
                        accum_out=sS[w % 2][:, si, m:m + 1])
            # s finalize: reduce groups, fold block-parity halves (via a
            # cross-partition DMA swap; both-SBUF ops can't cross), scale
            r = rS[w % 2]
            r2 = rSw[w % 2]
            nc.vector.reduce_sum(r[:, 0:2], sS[w % 2][:, :, :],
                                 axis=mybir.AxisListType.X)
            nc.gpsimd.tensor_copy(r2[0:64, 0:2], r[64:128, 0:2])
            nc.gpsimd.tensor_copy(r2[64:128, 0:2], r[0:64, 0:2])
            nc.vector.tensor_tensor(out=r2[:, 0:2], in0=r[:, 0:2],
                                    in1=r2[:, 0:2], op=ALU.add)
            # sgam = gam * [s_t (col 0, lo) | s_t1 (col 1, hi)]
            nc.vector.tensor_scalar_mul(sgam[w % 2][0:64, 0:1],
                                        r2[0:64, 0:1], gam[0:64])
            nc.vector.tensor_scalar_mul(sgam[w % 2][64:128, 0:1],
                                        r2[64:128, 1:2], gam[64:128])
        if w > 0:
            for j in range(NBLK):
                fs(w - 1, j)
                if j == 3:
                    store_half(w - 1, True)
            store_half(w - 1, False)


_CACHE = {}


def _build():
    if "nc" in _CACHE:
        return _CACHE["nc"]
    nc = bacc.Bacc("TRN2", target_bir_lowering=False, debug=False,
                   enable_asserts=False, num_devices=8)
    x_d = nc.dram_tensor("x", (C, T, W, H), BF16, kind="ExternalInput").ap()
    w_d = nc.dram_tensor("wpack", (128, 9, 192), BF16,
                         kind="ExternalInput").ap()
    biases_d = nc.dram_tensor("biases", (128, 4), F32,
                              kind="ExternalInput").ap()
    out_d = nc.dram_tensor("out", (C, T, W, H), BF16,
                           kind="ExternalOutput").ap()
    with tile.TileContext(nc) as tc, ExitStack() as ctx:
        _emit(nc, tc, x_d, w_d, biases_d, out_d, ctx)
    nc.compile()
    _CACHE["nc"] = nc
    return nc


def run_spmd(x, wq, wk, wv, bq, bk, bv, gamma, trace=False, **kw):
    nc = _build()
    wp = _pack_weights(wq, wk, wv)
    biases = np.zeros((128, 4), np.float32)
    for h in (0, 64):
        biases[h:h + 64, 0] = np.asarray(bq, np.float32)
        biases[h:h + 64, 1] = np.asarray(bk, np.float32)
        biases[h:h + 64, 2] = np.asarray(bv, np.float32)
    biases[:, 3] = np.float32(np.asarray(gamma).reshape(-1)[0])
    x = np.asarray(x, np.float32)
    in_maps = [
        {"x": np.ascontiguousarray(x[b]).astype(BF), "wpack": wp,
         "biases": biases}
        for b in range(B)
    ]
    res = bass_utils.run_bass_kernel_spmd(
        nc, in_maps, core_ids=list(range(B)), trace=trace, **kw)
    out = np.stack([np.asarray(res.results[b]["out"], np.float32)
                    for b in range(B)], axis=0)
    return out, res


def kernel(x, wq, wk, wv, bq, bk, bv, gamma):
    out, _ = run_spmd(x, wq, wk, wv, bq, bk, bv, gamma)
    return out
